# revision 2
# baseline (speedup 1.0000x reference)
"""Trainium2 Bass kernel for ContextQueryAttention — 1-core, engine-balanced.

Measurement model (established empirically for this axon-tunneled harness):
  per-call = dispatch floor (~1.2 ms 1-device vs ~3 ms 8-device) + device time;
  large-output-tensor overheads hit multi-device executes only. So: ONE core,
  ONE packed input tensor, ONE bf16 output tensor, minimal device time.

Device-time structure (vs the naive version):
  * exp outputs (softmax numerators E) in bf16 -> all consuming matmuls run
    full-rate at any moving size (f32r needs >=256)
  * scores stay f32r/f32 (PSUM) for accuracy; tolerance 2e-2 >> bf16 out err
  * only Q is pre-scaled by w_qc; it serves as matmul rhs for the X path and
    as stationary lhsT for the X^T path (saves scaling C: 1024 DVE el/batch)
  * ones-columns of the transposed tiles are written once (persistent manual
    double buffers), not per batch
  * per-k output normalization merges A|Bt into one tensor_scalar op
  * small PSUM->SBUF staging copies (softmax score biases) ride on the Act
    engine, which has headroom; DVE keeps the bulk converts
  * output DRAM layout [b, 2, p, k, d] gives 2KB contiguous DMA lines
"""

import numpy as np

import concourse.bass as bass
import concourse.bacc as bacc
import concourse.tile as tile
from concourse import mybir
from concourse.bass_utils import run_bass_kernel_spmd
from concourse.masks import make_identity

F32 = mybir.dt.float32
F32R = mybir.dt.float32r
BF16 = mybir.dt.bfloat16

B, D, N, M = 64, 128, 1024, 256
NCORES = 1
BPC = B
NK = N // 128
MJ = M // 128
FIN = N + M + 5  # packed input free size per batch


def build_kernel(bpc: int = BPC) -> bass.Bass:
    nc = bacc.Bacc("TRN2", target_bir_lowering=False, debug=False)

    IN8 = nc.dram_tensor("IN", [bpc, D, FIN], F32, kind="ExternalInput").ap()
    # A = OUT[:, 0, p, k, :] -> A[b, k*128+p, :]; Bt likewise at OUT[:, 1]
    OUT = nc.dram_tensor("OUT", [bpc, 2, 128, NK, D], BF16, kind="ExternalOutput").ap()

    with tile.TileContext(nc) as tc:
        with (
            tc.tile_pool(name="singles", bufs=1) as singles,
            tc.tile_pool(name="inp", bufs=2) as pool_in,
            tc.tile_pool(name="scaled", bufs=2) as pool_sc,
            tc.tile_pool(name="e", bufs=2) as pool_e,
            tc.tile_pool(name="small", bufs=2) as pool_sm,
            tc.tile_pool(name="out", bufs=2) as pool_out,
            tc.tile_pool(name="pp_t", bufs=1, space="PSUM") as pp_t,
            tc.tile_pool(name="pp_x", bufs=2, space="PSUM") as pp_x,
            tc.tile_pool(name="pp_ab", bufs=2, space="PSUM") as pp_ab,
            tc.tile_pool(name="pp_xt", bufs=2, space="PSUM") as pp_xt,
            tc.tile_pool(name="pp_g", bufs=1, space="PSUM") as pp_g,
        ):
            # w-cols packed in the input: (w_q, w_q, w_c, w_c, w_qc)
            wvec = singles.tile([D, 5], F32)
            nc.sync.dma_start(out=wvec, in_=IN8[0, :, N + M : N + M + 5])
            wq2 = singles.tile([D, 2], F32R)
            nc.sync.dma_start(
                out=wq2, in_=IN8[0, :, N + M : N + M + 2].bitcast(F32R)
            )
            w_qc = wvec[:, 4:5]
            ones2 = singles.tile([128, 2], BF16)
            nc.vector.memset(ones2, 1.0)
            ident_f32 = singles.tile([128, 128], F32)
            make_identity(nc, ident_f32)
            ident = singles.tile([128, 128], F32R)
            nc.vector.tensor_copy(out=ident, in_=ident_f32)

            # persistent manual double-buffers for the transposed tiles, so the
            # all-ones columns are written once, not per batch
            ct_bufs = [singles.tile([128, NK, D + 2], BF16, name=f"ctb{i}") for i in range(2)]
            qtg_bufs = [singles.tile([128, MJ, 2 * D + 2], BF16, name=f"qtgb{i}") for i in range(2)]
            for t in ct_bufs:
                for k in range(NK):
                    nc.vector.tensor_copy(out=t[:, k, D : D + 2], in_=ones2)
            for t in qtg_bufs:
                for j in range(MJ):
                    nc.vector.tensor_copy(out=t[:, j, D : D + 2], in_=ones2)

            for b in range(bpc):
                ct = ct_bufs[b % 2]
                qtg = qtg_bufs[b % 2]

                cq = pool_in.tile([D, N + M], F32R, tag="cq")
                nc.sync.dma_start(out=cq, in_=IN8[b, :, 0 : N + M].bitcast(F32R))
                cb = cq[:, 0:N]
                qb = cq[:, N : N + M]

                # qswc = [Q * w_qc | w_c w_c]  (rhs for X, lhsT for X^T)
                qswc = pool_sc.tile([D, M + 2], F32R, tag="qswc")
                nc.vector.tensor_scalar_mul(out=qswc[:, 0:M], in0=qb, scalar1=w_qc)
                nc.vector.tensor_copy(out=qswc[:, M : M + 2], in_=wvec[:, 2:4])

                # transposes: ct_k[:, 0:D] = Ct_k, qtg_j[:, 0:D] = Qt_j
                for k in range(NK):
                    pt = pp_t.tile([128, 128], F32R, tag="pt")
                    nc.tensor.transpose(pt, cb[:, k * 128 : (k + 1) * 128], ident)
                    nc.vector.tensor_copy(out=ct[:, k, 0:D], in_=pt)
                for j in range(MJ):
                    pt = pp_t.tile([128, 128], F32R, tag="pt")
                    nc.tensor.transpose(pt, qb[:, j * 128 : (j + 1) * 128], ident)
                    nc.vector.tensor_copy(out=qtg[:, j, 0:D], in_=pt)

                # X [n,m] chunks + fused cscore -> E_col = exp(X + cs) bf16
                e_col = pool_e.tile([128, NK, M], BF16, tag="e_col")
                for k in range(NK):
                    px = pp_x.tile([128, M + 2], F32, tag="px")
                    nc.tensor.matmul(
                        px, cb[:, k * 128 : (k + 1) * 128], qswc, start=True, stop=True
                    )
                    cs_k = pool_sm.tile([128, 1], F32, tag=f"cs{k}")
                    nc.scalar.copy(out=cs_k, in_=px[:, M : M + 1])
                    nc.scalar.activation(
                        out=e_col[:, k, :],
                        in_=px[:, 0:M],
                        func=mybir.ActivationFunctionType.Exp,
                        bias=cs_k,
                    )

                # X^T [m,n] chunks (lhsT = scaled Q) + qscore -> E_row bf16
                e_row = pool_e.tile([128, MJ, N], BF16, tag="e_row")
                for j in range(MJ):
                    qsj = qswc[:, j * 128 : (j + 1) * 128]
                    pq = pp_g.tile([128, D + 2], F32, tag="pg")
                    nc.tensor.matmul(
                        pq[:, 0:2],
                        qb[:, j * 128 : (j + 1) * 128],
                        wq2,
                        start=True,
                        stop=True,
                    )
                    qs_j = pool_sm.tile([128, 1], F32, tag=f"qs{j}")
                    nc.scalar.copy(out=qs_j, in_=pq[:, 0:1])
                    for h in range(N // 512):
                        pxt = pp_xt.tile([128, 512], F32, tag="pxt")
                        nc.tensor.matmul(
                            pxt,
                            qsj,
                            cb[:, h * 512 : (h + 1) * 512],
                            start=True,
                            stop=True,
                        )
                        nc.scalar.activation(
                            out=e_row[:, j, h * 512 : (h + 1) * 512],
                            in_=pxt,
                            func=mybir.ActivationFunctionType.Exp,
                            bias=qs_j,
                        )

                # col path: G_j = normalize(E^T @ [Ct|1 1])  (bf16, full rate)
                for j in range(MJ):
                    pg = pp_g.tile([128, D + 2], F32, tag="pg")
                    for k in range(NK):
                        nc.tensor.matmul(
                            pg,
                            e_col[:, k, j * 128 : (j + 1) * 128],
                            ct[:, k, :],
                            start=(k == 0),
                            stop=(k == NK - 1),
                        )
                    rcol = pool_sm.tile([128, 1], F32, tag=f"rcol{j}")
                    nc.vector.reciprocal(out=rcol, in_=pg[:, D : D + 1])
                    nc.vector.tensor_scalar_mul(
                        out=qtg[:, j, D + 2 : 2 * D + 2], in0=pg[:, 0:D], scalar1=rcol
                    )

                # row path: [A | rowsum rowsum | Bt] = E'^T @ [Qt|1 1|G];
                # one merged normalization per k into oAB
                oAB = pool_out.tile([128, NK, 2 * D + 2], BF16, tag="oAB")
                for k in range(NK):
                    pab = pp_ab.tile([128, 2 * D + 2], F32, tag="pab")
                    for j in range(MJ):
                        nc.tensor.matmul(
                            pab,
                            e_row[:, j, k * 128 : (k + 1) * 128],
                            qtg[:, j, :],
                            start=(j == 0),
                            stop=(j == MJ - 1),
                        )
                    rrow = pool_sm.tile([128, 1], F32, tag=f"rrow{k}")
                    nc.vector.reciprocal(out=rrow, in_=pab[:, D : D + 1])
                    nc.vector.tensor_scalar_mul(
                        out=oAB[:, k, :], in0=pab, scalar1=rrow
                    )
                nc.sync.dma_start(out=OUT[b, 0], in_=oAB[:, :, 0:D])
                nc.sync.dma_start(out=OUT[b, 1], in_=oAB[:, :, D + 2 : 2 * D + 2])
    nc.finalize()
    return nc


def pack_inputs(C, Q, W0):
    """[B, D, N+M+5] f32: per batch [C | Q | w-cols]."""
    Bn = C.shape[0]
    wt = np.asarray(W0, dtype=np.float32).reshape(3, D).T  # [D, 3]
    wcols = wt[:, [0, 0, 1, 1, 2]]  # (w_q, w_q, w_c, w_c, w_qc)
    IN = np.empty((Bn, D, FIN), dtype=np.float32)
    IN[:, :, 0:N] = C
    IN[:, :, N : N + M] = Q
    IN[:, :, N + M :] = wcols[None]
    return IN


def unpack_outputs(out):
    """OUT [B, 2, 128, NK, D] bf16 -> (A, Bt) [B, N, D] f32 each."""
    out = np.asarray(out, dtype=np.float32)
    # n = k*128 + p  ->  [B, 2, k, p, d] then merge (k p)
    out = out.transpose(0, 1, 3, 2, 4).reshape(out.shape[0], 2, N, D)
    return out[:, 0], out[:, 1]


def make_in_maps(inputs, ncores=NCORES):
    IN = pack_inputs(
        np.asarray(inputs["C"], np.float32),
        np.asarray(inputs["Q"], np.float32),
        np.asarray(inputs["W0"], np.float32),
    )
    bpc = B // ncores
    return [{"IN": IN[i * bpc : (i + 1) * bpc]} for i in range(ncores)]


_NC_CACHE = None


def kernel(C, Q, W0, b0, _trace=False):
    global _NC_CACHE
    if _NC_CACHE is None:
        _NC_CACHE = build_kernel()
    nc = _NC_CACHE

    IN = pack_inputs(
        np.asarray(C, np.float32), np.asarray(Q, np.float32), np.asarray(W0, np.float32)
    )
    res = run_bass_kernel_spmd(nc, [{"IN": IN}], core_ids=[0])
    return unpack_outputs(res.results[0]["OUT"])
